# revision 20
# baseline (speedup 1.0000x reference)
"""Trainium2 Bass kernel for a 3-branch GCN layer (sum of three GCNConvs).

Math: out[b,t] = sum_k A_k @ (x[b,t] @ W_k) + b_k with A_k the normalized
adjacency (self loops) of tiny shared graphs, so the operator collapses to
one [1600 x 1600] block matrix Mop = sum_k kron(A_k^T, W_k) applied to x
rows. Mop is BLOCK-SPARSE: 64x64 block (m,n) is nonzero only when some
graph couples source node m to target node n (~30% density here).

Device-side design (data-parallel over batch across 8 cores):
  - x is cast to fp16 AND pre-transposed/tiled on the host into xT chunks
    [128 (m-pair, d), 480 (bt)] so the device does ZERO transposes.
  - Flipped GEMM: out^T[(n,c), bt] accumulates in PSUM; the Mop blocks are
    PE-stationary, xT chunks stream 480 bt-columns (hides LDWEIGHTS). Only
    NONZERO blocks are stored/loaded/streamed.
  - Output nodes are matched into fixed pairs sharing one [128, 480] PSUM
    tile. A chunk coupling BOTH nodes of a pair runs as one full-width
    M=128 matmul; single-coupled chunks run as M=64 matmuls, interleaved
    top/bottom so PE column tiling ((0,0)/(0,64)) executes two at once.
  - Source m-pairs and target n-pairs are jointly optimized (matching) to
    minimize nonzero blocks. Host un-permutes out^T and adds bias.
  - DMA is batched: one xT load + one Mop piece + two out stores per
    group, so DGE issue cost (~0.6us each) stays off the critical path.
"""

import itertools
import sys

import numpy as np

if "/opt/trn_rl_repo" not in sys.path:
    sys.path.insert(0, "/opt/trn_rl_repo")

B, T, NNODES, C = 64, 300, 25, 64
F = NNODES * C  # 1600
N_CORES = 8
BT_LOC = (B // N_CORES) * T  # 2400
NGRP = 5
GCOL = BT_LOC // NGRP  # 480 bt-columns per group
NPAIR = 13  # 25 nodes + 1 zero-pad node -> 13 m-pair chunks of K=128
NTILE = 13  # 12 n-pairs + 1 single-n psum tile

_PROGRAM_CACHE = {}
# extra kwargs for run_bass_kernel_spmd (test harness sets trace=True here)
_RUN_KW = {}


def _dense_adj(edge_index_k: np.ndarray) -> np.ndarray:
    """PyG GCNConv normalized dense adjacency A[dst, src] (float64)."""
    row = edge_index_k[0].astype(np.int64)
    col = edge_index_k[1].astype(np.int64)
    loop = np.arange(NNODES, dtype=np.int64)
    row = np.concatenate([row, loop])
    col = np.concatenate([col, loop])
    deg = np.zeros(NNODES, dtype=np.float64)
    np.add.at(deg, col, 1.0)
    dinv = np.where(deg > 0, 1.0 / np.sqrt(deg), 0.0)
    norm = dinv[row] * dinv[col]
    A = np.zeros((NNODES, NNODES), dtype=np.float64)
    np.add.at(A, (col, row), norm)
    return A


def _match(items, weight):
    """Max-weight perfect-ish matching; networkx blossom with greedy
    fallback. Returns (pairs, leftover_single)."""
    items = list(items)
    try:
        import networkx as nx

        G = nx.Graph()
        G.add_nodes_from(items)
        for a, b in itertools.combinations(items, 2):
            G.add_edge(a, b, weight=weight(a, b))
        M = nx.max_weight_matching(G, maxcardinality=True)
        pairs = [tuple(sorted(p)) for p in M]
    except Exception:
        rem = set(items)
        pairs = []
        while len(rem) > 1:
            rl = sorted(rem)
            best, bv = None, -(10**9)
            for i, a in enumerate(rl):
                for b in rl[i + 1:]:
                    v = weight(a, b)
                    if v > bv:
                        bv, best = v, (a, b)
            pairs.append(best)
            rem -= set(best)
    used = set(x for p in pairs for x in p)
    single = [x for x in items if x not in used]
    return sorted(pairs), (single[0] if single else None)


def _cover(n, srcs_n, chunks):
    """Min chunks covering target n's source set: matched pairs (both
    sources in one chunk) + leftover singles. Returns {chunk_idx:
    set(active sources)}."""
    usable = {}
    for i, (a, b) in enumerate(chunks):
        if b is not None and a in srcs_n and b in srcs_n:
            usable.setdefault((a, b), i)
    try:
        import networkx as nx

        G = nx.Graph()
        G.add_nodes_from(srcs_n)
        for (a, b) in usable:
            G.add_edge(a, b)
        M = [tuple(sorted(p)) for p in nx.max_weight_matching(G)]
    except Exception:
        M = []
        rem = set(srcs_n)
        for (a, b) in sorted(usable):
            if a in rem and b in rem:
                M.append((a, b))
                rem -= {a, b}
    cov = {}
    used = set()
    for (a, b) in M:
        cov[usable[(a, b)]] = {a, b}
        used |= {a, b}
    for m in sorted(set(srcs_n) - used):
        ci = next(i for i, c in enumerate(chunks) if m in (c[0], c[1]))
        cov.setdefault(ci, set()).add(m)
    return cov


def _plan(As, max_chunks=21):
    """Choose source-pair chunks (hot sources may repeat across chunks),
    per-target covers, and target n-pairs, minimizing PE windows."""
    U = np.zeros((NNODES, NNODES), dtype=bool)
    for A in As:
        U |= A != 0.0  # U[n, m]: target n couples to source m
    srcs = [set(np.where(U[n])[0]) for n in range(NNODES)]

    # base chunks: matching maximizing shared target sets
    cols = [set(np.where(U[:, m])[0]) for m in range(NNODES)]
    mpairs, msingle = _match(range(NNODES), lambda a, b: len(cols[a] & cols[b]))
    chunks = mpairs + [(msingle, None)]

    # greedily duplicate hot source pairs into extra chunks while it
    # keeps reducing per-target coverage (fewer matmul windows); only
    # targets containing both candidate sources can improve
    covn = [len(_cover(n, srcs[n], chunks)) for n in range(NNODES)]
    while len(chunks) < max_chunks:
        best, bv = None, 0
        for a, b in itertools.combinations(range(NNODES), 2):
            if (a, b) in chunks:
                continue
            aff = [n for n in range(NNODES) if a in srcs[n] and b in srcs[n]]
            gain = sum(
                covn[n] - len(_cover(n, srcs[n], chunks + [(a, b)]))
                for n in aff
            )
            if gain > bv:
                bv, best = gain, (a, b)
        if best is None or bv < 2:
            break
        chunks.append(best)
        covn = [len(_cover(n, srcs[n], chunks)) for n in range(NNODES)]

    covs = [_cover(n, srcs[n], chunks) for n in range(NNODES)]
    rows = [set(c.keys()) for c in covs]

    # n-pairing: tile window count = max(|Ra|,|Rb|); overlap tie-break
    npairs, nsingle = _match(
        range(NNODES),
        lambda a, b: -3 * max(len(rows[a]), len(rows[b]))
        + len(rows[a] & rows[b]),
    )
    ncl = npairs + [(nsingle, None)]

    tiles = []
    tdelta = 0  # running (top singles - bottom singles)
    for na, nb in ncl:
        ra = rows[na]
        rb = rows[nb] if nb is not None else set()
        sa = sorted(ra - rb)
        sb = sorted(rb - ra)
        both = sorted(ra & rb)
        if nb is not None and (
            (tdelta > 0 and len(sa) > len(sb))
            or (tdelta < 0 and len(sb) > len(sa))
        ):
            na, nb, sa, sb = nb, na, sb, sa
        tdelta += len(sa) - len(sb)
        tiles.append({"ntop": na, "nbot": nb, "both": both, "top": sa, "bot": sb})
    return {"chunks": chunks, "covs": covs, "tiles": tiles, "U": U}


def _mop_blocks(plan, As, Ws):
    """Pack nonzero Mop blocks fp16 and build the matmul schedule.

    Returns (mopb [128, TOTCOL] f16, sched) with sched[j] = list of
    (col_off, width, row_base, chunk_idx, start, stop)."""
    Wstack = np.stack(Ws)  # [3, 64, 64] float64

    def half_block(n, I):
        # only sources ASSIGNED to chunk I for target n contribute; a
        # source present in the chunk but covered elsewhere stays zero
        blk = np.zeros((128, C), dtype=np.float64)
        active = plan["covs"][n].get(I, set())
        for h, m in enumerate(plan["chunks"][I]):
            if m is not None and m in active:
                coef = np.array([A[n, m] for A in As])
                blk[h * C:(h + 1) * C] = np.tensordot(coef, Wstack, 1)
        return blk

    cols = []
    off = 0
    sched = []
    last_kind = "b"  # chain window kinds across tile boundaries
    for t in plan["tiles"]:
        boths, singles = [], []
        for I in t["both"]:
            cols.append(half_block(t["ntop"], I))
            cols.append(half_block(t["nbot"], I))
            boths.append([off, 128, 0, I])
            off += 128
        tops, bots = [], []
        for base, key, lst in ((0, "top", tops), (C, "bot", bots)):
            for I in t[key]:
                cols.append(half_block(t["ntop"] if base == 0 else t["nbot"], I))
                lst.append([off, C, base, I])
                off += C
        inter = [e for pair in itertools.zip_longest(tops, bots) for e in pair
                 if e is not None]
        singles = inter
        # orient so the tile starts with the same window kind the
        # previous tile ended with (LDWEIGHTS-bus spills happen at
        # both<->dual transitions)
        if last_kind == "b":
            seq = boths + singles
        else:
            seq = singles + boths
        if seq:
            last_kind = "b" if seq[-1][1] == 128 else "s"
        # start flags: first writer of each row region
        seen0 = seen64 = False
        out = []
        for q, (o, w, rb, I) in enumerate(seq):
            regions = (0, 1) if w == 128 else ((0,) if rb == 0 else (1,))
            start = (0 in regions and not seen0) or (1 in regions and not seen64)
            if 0 in regions:
                seen0 = True
            if 1 in regions:
                seen64 = True
            out.append((o, w, rb, I, start, q == len(seq) - 1))
        sched.append(out)
    mopb = np.concatenate(cols, axis=1).astype(np.float16)
    return np.ascontiguousarray(mopb), sched


def _chunk_order(sched, nchunks):
    """Chunk slots ordered by first use, so the first xT DMA piece covers
    the chunks the early tiles need."""
    order = []
    for entries in sched:
        for (_, _, _, I, _, _) in entries:
            if I not in order:
                order.append(I)
    order += [i for i in range(nchunks) if i not in order]
    return order


def _build_program(sched, slot_of, totcol, nchunks):
    import concourse.bass as bass
    import concourse.tile as tile
    from concourse import bacc, mybir

    f32 = mybir.dt.float32
    f16 = mybir.dt.float16

    nc = bacc.Bacc(
        "TRN2", target_bir_lowering=False, debug=False, num_devices=N_CORES
    )
    xt = nc.dram_tensor(
        "xt", [NGRP, 128, nchunks * GCOL], f16, kind="ExternalInput"
    ).ap()
    mop = nc.dram_tensor("mopb", [128, totcol], f16, kind="ExternalInput").ap()
    outt = nc.dram_tensor(
        "outt", [NGRP, 128, NTILE * GCOL], f16, kind="ExternalOutput"
    ).ap()

    # split points: xT pieces by first use (small first piece so the
    # first matmuls start early); mop likewise
    XSPLS = sorted(set([0, 2 * GCOL, 4 * GCOL, 7 * GCOL, 10 * GCOL,
                        14 * GCOL, nchunks * GCOL]))
    MSPLS = sorted(
        set([0] + [totcol * k // 6 // 2 * 2 for k in range(1, 6)] + [totcol])
    )
    OSPL = [0, 5 * GCOL, 9 * GCOL, 12 * GCOL, NTILE * GCOL]

    with tile.TileContext(nc) as tc:
        with (
            tc.tile_pool(name="const", bufs=1) as const_pool,
            tc.tile_pool(name="xg", bufs=2) as xg_pool,
            tc.tile_pool(name="outp", bufs=2) as out_pool,
            tc.tile_pool(name="ps", bufs=7, space="PSUM") as ps_pool,
            tc.tile_pool(name="wm", bufs=1, space="PSUM") as warm_pool,
        ):
            # warmup: dummy matmuls on an uninitialized SBUF tile keep the
            # PE busy through the HAM activity window while the first DMA
            # pieces land, so the real matmuls run at 2.4 GHz from the
            # start instead of 1.2 GHz for their first ~10us
            wsrc = const_pool.tile([128, 512], f16, tag="warm")
            nc.gpsimd.memset(wsrc[:], 0)
            wps = warm_pool.tile([128, 512], f32, tag="wps")

            def dummy_mms(k):
                # keep the PE busy through startup DMA waits: heats the
                # HAM activity window and prevents a mid-start re-throttle
                for _ in range(k):
                    nc.tensor.matmul(
                        wps[:], wsrc[:, 0:128], wsrc[:],
                        start=True, stop=True, skip_group_check=True,
                    )

            dummy_mms(10)

            mop_sb = const_pool.tile([128, totcol], f16, tag="mop")
            for a, b in zip(MSPLS, MSPLS[1:]):
                nc.gpsimd.dma_start(mop_sb[:, a:b], mop[:, a:b])

            for g in range(NGRP):
                xg = xg_pool.tile([128, nchunks * GCOL], f16, tag="x")
                for a, b in zip(XSPLS, XSPLS[1:]):
                    nc.sync.dma_start(xg[:, a:b], xt[g, :, a:b])
                ot = out_pool.tile([128, NTILE * GCOL], f16, tag="o")
                for j, entries in enumerate(sched):
                    ps = ps_pool.tile([128, GCOL], f32, tag="ps")
                    for (off, w, rb, I, st, sp) in entries:
                        s = slot_of[I] * GCOL
                        nc.tensor.matmul(
                            ps[rb:rb + w, :],
                            mop_sb[:, off:off + w],
                            xg[:, s:s + GCOL],
                            start=st,
                            stop=sp,
                            tile_position=(0, rb),
                            skip_group_check=True,
                        )
                    dst = ot[:, j * GCOL:(j + 1) * GCOL]
                    if j % 2 == 0:
                        nc.vector.tensor_copy(dst, ps[:])
                    else:
                        nc.scalar.copy(dst, ps[:])
                    for a, b in zip(OSPL, OSPL[1:]):
                        if (j + 1) * GCOL == b:
                            nc.sync.dma_start(
                                outt[g, :, a:b], ot[:, a:b]
                            )
                    if g == 0 and j < 6:
                        dummy_mms(3)

    nc.compile()
    return nc


def kernel(x, edge_index, W1, W2, W3, b1, b2, b3):
    from concourse.bass_utils import run_bass_kernel_spmd

    x = np.asarray(x, dtype=np.float32)
    edge_index = np.asarray(edge_index)
    Ws = [np.asarray(W, dtype=np.float64) for W in (W1, W2, W3)]
    bs = [np.asarray(b, dtype=np.float64) for b in (b1, b2, b3)]

    As = [_dense_adj(edge_index[k]) for k in range(3)]
    plan = _plan(As)
    mopb, sched = _mop_blocks(plan, As, Ws)
    nchunks = len(plan["chunks"])
    order = _chunk_order(sched, nchunks)
    slot_of = {I: s for s, I in enumerate(order)}
    totcol = mopb.shape[1]

    key = str(sched) + str(order) + str(nchunks)
    if _PROGRAM_CACHE.get("key") != key:
        _PROGRAM_CACHE["nc"] = _build_program(sched, slot_of, totcol, nchunks)
        _PROGRAM_CACHE["key"] = key
    nc = _PROGRAM_CACHE["nc"]

    # host-side prep: fp16 cast + transpose + chunk packing in slot
    # (first-use) order; hot sources may appear in several chunks
    x16 = x.astype(np.float16)
    xr = x16.reshape(N_CORES, NGRP, GCOL, NNODES, C).transpose(0, 1, 3, 4, 2)
    xr = np.concatenate(
        [xr, np.zeros((N_CORES, NGRP, 1, C, GCOL), dtype=np.float16)], axis=2
    )  # zero-pad node index 25
    pidx = np.array(
        [
            (m if m is not None else NNODES)
            for I in order
            for m in plan["chunks"][I]
        ]
    )
    # [cores, grp, 26, 64, gcol] -> [cores, grp, 128(slot-major), ...]
    xtil = xr[:, :, pidx].reshape(N_CORES, NGRP, nchunks, 128, GCOL)
    xtil = np.ascontiguousarray(
        xtil.transpose(0, 1, 3, 2, 4).reshape(
            N_CORES, NGRP, 128, nchunks * GCOL
        )
    )

    in_maps = [{"xt": xtil[i], "mopb": mopb} for i in range(N_CORES)]
    res = run_bass_kernel_spmd(nc, in_maps, list(range(N_CORES)), **_RUN_KW)
    _PROGRAM_CACHE["last_result"] = res

    bias = np.zeros(C, dtype=np.float64)
    for b in bs:
        bias += b
    out = np.empty((N_CORES, BT_LOC, NNODES, C), dtype=np.float32)
    for i in range(N_CORES):
        # [grp, 128, NTILE*gcol] -> [grp, 128, NTILE, gcol]
        ot = (
            res.results[i]["outt"]
            .reshape(NGRP, 128, NTILE, GCOL)
            .astype(np.float32)
        )
        for j, t in enumerate(plan["tiles"]):
            out[i, :, t["ntop"]] = (
                ot[:, 0:C, j].transpose(0, 2, 1).reshape(BT_LOC, C)
            )
            if t["nbot"] is not None:
                out[i, :, t["nbot"]] = (
                    ot[:, C:128, j].transpose(0, 2, 1).reshape(BT_LOC, C)
                )
    out += bias.astype(np.float32)
    return np.ascontiguousarray(
        out.reshape(B, T, NNODES, C).astype(np.float32)
    )


# revision 25
# speedup vs baseline: 1.0503x; 1.0503x over previous
"""Trainium2 Bass kernel for a 3-branch GCN layer (sum of three GCNConvs).

Math: out[b,t] = sum_k A_k @ (x[b,t] @ W_k) + b_k with A_k the normalized
adjacency (self loops) of tiny shared graphs, so the operator collapses to
one [1600 x 1600] block matrix Mop = sum_k kron(A_k^T, W_k) applied to x
rows. Mop is BLOCK-SPARSE: 64x64 block (m,n) is nonzero only when some
graph couples source node m to target node n (~30% density here).

Device-side design (data-parallel over batch across 8 cores):
  - x is cast to fp16 AND pre-transposed/tiled on the host into xT chunks
    [128 (m-pair, d), 480 (bt)] so the device does ZERO transposes.
  - Flipped GEMM: out^T[(n,c), bt] accumulates in PSUM; the Mop blocks are
    PE-stationary, xT chunks stream 480 bt-columns (hides LDWEIGHTS). Only
    NONZERO blocks are stored/loaded/streamed.
  - Output nodes are matched into fixed pairs sharing one [128, 480] PSUM
    tile. A chunk coupling BOTH nodes of a pair runs as one full-width
    M=128 matmul; single-coupled chunks run as M=64 matmuls, interleaved
    top/bottom so PE column tiling ((0,0)/(0,64)) executes two at once.
  - Source m-pairs and target n-pairs are jointly optimized (matching) to
    minimize nonzero blocks. Host un-permutes out^T and adds bias.
  - DMA is batched: one xT load + one Mop piece + two out stores per
    group, so DGE issue cost (~0.6us each) stays off the critical path.
"""

import itertools
import sys

import numpy as np

if "/opt/trn_rl_repo" not in sys.path:
    sys.path.insert(0, "/opt/trn_rl_repo")

B, T, NNODES, C = 64, 300, 25, 64
F = NNODES * C  # 1600
N_CORES = 8
BT_LOC = (B // N_CORES) * T  # 2400
NGRP = 5
GCOL = BT_LOC // NGRP  # 480 bt-columns per group
NPAIR = 13  # 25 nodes + 1 zero-pad node -> 13 m-pair chunks of K=128
NTILE = 13  # 12 n-pairs + 1 single-n psum tile

_PROGRAM_CACHE = {}
# extra kwargs for run_bass_kernel_spmd (test harness sets trace=True here)
_RUN_KW = {}


def _dense_adj(edge_index_k: np.ndarray) -> np.ndarray:
    """PyG GCNConv normalized dense adjacency A[dst, src] (float64)."""
    row = edge_index_k[0].astype(np.int64)
    col = edge_index_k[1].astype(np.int64)
    loop = np.arange(NNODES, dtype=np.int64)
    row = np.concatenate([row, loop])
    col = np.concatenate([col, loop])
    deg = np.zeros(NNODES, dtype=np.float64)
    np.add.at(deg, col, 1.0)
    dinv = np.where(deg > 0, 1.0 / np.sqrt(deg), 0.0)
    norm = dinv[row] * dinv[col]
    A = np.zeros((NNODES, NNODES), dtype=np.float64)
    np.add.at(A, (col, row), norm)
    return A


def _match(items, weight):
    """Max-weight perfect-ish matching; networkx blossom with greedy
    fallback. Returns (pairs, leftover_single)."""
    items = list(items)
    try:
        import networkx as nx

        G = nx.Graph()
        G.add_nodes_from(items)
        for a, b in itertools.combinations(items, 2):
            G.add_edge(a, b, weight=weight(a, b))
        M = nx.max_weight_matching(G, maxcardinality=True)
        pairs = [tuple(sorted(p)) for p in M]
    except Exception:
        rem = set(items)
        pairs = []
        while len(rem) > 1:
            rl = sorted(rem)
            best, bv = None, -(10**9)
            for i, a in enumerate(rl):
                for b in rl[i + 1:]:
                    v = weight(a, b)
                    if v > bv:
                        bv, best = v, (a, b)
            pairs.append(best)
            rem -= set(best)
    used = set(x for p in pairs for x in p)
    single = [x for x in items if x not in used]
    return sorted(pairs), (single[0] if single else None)


def _cover(n, srcs_n, chunks):
    """Min chunks covering target n's source set: matched pairs (both
    sources in one chunk) + leftover singles. Returns {chunk_idx:
    set(active sources)}."""
    usable = {}
    for i, (a, b) in enumerate(chunks):
        if b is not None and a in srcs_n and b in srcs_n:
            usable.setdefault((a, b), i)
    try:
        import networkx as nx

        G = nx.Graph()
        G.add_nodes_from(srcs_n)
        for (a, b) in usable:
            G.add_edge(a, b)
        M = [tuple(sorted(p)) for p in nx.max_weight_matching(G)]
    except Exception:
        M = []
        rem = set(srcs_n)
        for (a, b) in sorted(usable):
            if a in rem and b in rem:
                M.append((a, b))
                rem -= {a, b}
    cov = {}
    used = set()
    for (a, b) in M:
        cov[usable[(a, b)]] = {a, b}
        used |= {a, b}
    for m in sorted(set(srcs_n) - used):
        ci = next(i for i, c in enumerate(chunks) if m in (c[0], c[1]))
        cov.setdefault(ci, set()).add(m)
    return cov


def _plan(As, max_chunks=21):
    """Choose source-pair chunks (hot sources may repeat across chunks),
    per-target covers, and target n-pairs, minimizing PE windows."""
    U = np.zeros((NNODES, NNODES), dtype=bool)
    for A in As:
        U |= A != 0.0  # U[n, m]: target n couples to source m
    srcs = [set(np.where(U[n])[0]) for n in range(NNODES)]

    # base chunks: matching maximizing shared target sets
    cols = [set(np.where(U[:, m])[0]) for m in range(NNODES)]
    mpairs, msingle = _match(range(NNODES), lambda a, b: len(cols[a] & cols[b]))
    chunks = mpairs + [(msingle, None)]

    # greedily duplicate hot source pairs into extra chunks while it
    # keeps reducing per-target coverage (fewer matmul windows); only
    # targets containing both candidate sources can improve
    covn = [len(_cover(n, srcs[n], chunks)) for n in range(NNODES)]
    while len(chunks) < max_chunks:
        best, bv = None, 0
        for a, b in itertools.combinations(range(NNODES), 2):
            if (a, b) in chunks:
                continue
            aff = [n for n in range(NNODES) if a in srcs[n] and b in srcs[n]]
            gain = sum(
                covn[n] - len(_cover(n, srcs[n], chunks + [(a, b)]))
                for n in aff
            )
            if gain > bv:
                bv, best = gain, (a, b)
        if best is None or bv < 2:
            break
        chunks.append(best)
        covn = [len(_cover(n, srcs[n], chunks)) for n in range(NNODES)]

    covs = [_cover(n, srcs[n], chunks) for n in range(NNODES)]
    rows = [set(c.keys()) for c in covs]

    # n-pairing: tile window count = max(|Ra|,|Rb|); overlap tie-break
    npairs, nsingle = _match(
        range(NNODES),
        lambda a, b: -3 * max(len(rows[a]), len(rows[b]))
        + len(rows[a] & rows[b]),
    )
    ncl = npairs + [(nsingle, None)]

    tiles = []
    tdelta = 0  # running (top singles - bottom singles)
    for na, nb in ncl:
        ra = rows[na]
        rb = rows[nb] if nb is not None else set()
        sa = sorted(ra - rb)
        sb = sorted(rb - ra)
        both = sorted(ra & rb)
        if nb is not None and (
            (tdelta > 0 and len(sa) > len(sb))
            or (tdelta < 0 and len(sb) > len(sa))
        ):
            na, nb, sa, sb = nb, na, sb, sa
        tdelta += len(sa) - len(sb)
        tiles.append({"ntop": na, "nbot": nb, "both": both, "top": sa, "bot": sb})
    return {"chunks": chunks, "covs": covs, "tiles": tiles, "U": U}


def _mop_blocks(plan, As, Ws):
    """Pack nonzero Mop blocks fp16 and build the matmul schedule.

    Returns (mopb [128, TOTCOL] f16, sched) with sched[j] = list of
    (col_off, width, row_base, chunk_idx, start, stop)."""
    Wstack = np.stack(Ws)  # [3, 64, 64] float64

    def half_block(n, I):
        # only sources ASSIGNED to chunk I for target n contribute; a
        # source present in the chunk but covered elsewhere stays zero
        blk = np.zeros((128, C), dtype=np.float64)
        active = plan["covs"][n].get(I, set())
        for h, m in enumerate(plan["chunks"][I]):
            if m is not None and m in active:
                coef = np.array([A[n, m] for A in As])
                blk[h * C:(h + 1) * C] = np.tensordot(coef, Wstack, 1)
        return blk

    cols = []
    off = 0
    pertile = []
    for t in plan["tiles"]:
        boths = []
        for I in t["both"]:
            cols.append(half_block(t["ntop"], I))
            cols.append(half_block(t["nbot"], I))
            boths.append((off, 128, 0, I))
            off += 128
        tops, bots = [], []
        for base, key, lst in ((0, "top", tops), (C, "bot", bots)):
            for I in t[key]:
                cols.append(half_block(t["ntop"] if base == 0 else t["nbot"], I))
                lst.append((off, C, base, I))
                off += C
        pertile.append((boths, tops, bots))
    mopb = np.concatenate(cols, axis=1).astype(np.float16)

    # Global schedule: flat instruction stream of (tile_j, off, w, rb, I).
    # Within a tile, top/bottom singles interleave (PE column tiling runs
    # the pair concurrently); LEFTOVER solos are carried forward and
    # paired with the next tiles' opposite-side solos across tile
    # boundaries — different psum banks + different column groups still
    # run concurrently.
    stream = []
    pend = []  # carried solos: list of (j, entry)
    last_kind = "b"
    for j, (boths, tops, bots) in enumerate(pertile):
        tops = [(j, e) for e in tops]
        bots = [(j, e) for e in bots]
        # pair carried solos with our opposite side first
        mine = {0: tops, C: bots}
        pairs = []
        for (pj, pe) in list(pend):
            other = mine[C if pe[2] == 0 else 0]
            if other:
                pairs.append(((pj, pe), other.pop(0)))
                pend.remove((pj, pe))
        inner = [p for p in itertools.zip_longest(mine[0], mine[C])
                 if p[0] is not None and p[1] is not None]
        lead = [x for pr in (pairs + inner) for x in pr]
        new_solo = [x for x in mine[0] + mine[C]
                    if x not in [e for pr in inner for e in pr]]
        bj = [(j, e) for e in boths]
        if last_kind == "b":
            seq = bj + lead
        else:
            seq = lead + bj
        if seq:
            last_kind = "b" if seq[-1][1][1] == 128 else "s"
        stream.extend(seq)
        pend.extend(new_solo)
        # flush carried solos that can no longer pair (keep at most 4)
        while len(pend) > 4:
            stream.append(pend.pop(0))
    stream.extend(pend)

    # start/stop flags per (tile, region); drains wait on each tile's
    # final entry, so record per-tile order
    seen = {}
    lastidx = {}
    for q, (j, (o, w, rb, I)) in enumerate(stream):
        lastidx[j] = q
    sched = []
    for q, (j, (o, w, rb, I)) in enumerate(stream):
        regions = ((j, 0), (j, 1)) if w == 128 else (
            ((j, 0),) if rb == 0 else ((j, 1),)
        )
        start = any(r not in seen for r in regions)
        for r in regions:
            seen[r] = True
        sched.append((j, o, w, rb, I, start, q == lastidx[j]))
    return np.ascontiguousarray(mopb), sched


def _chunk_order(sched, nchunks):
    """Chunk slots ordered by first use, so the first xT DMA piece covers
    the chunks the early tiles need."""
    order = []
    for (_, _, _, _, I, _, _) in sched:
        if I not in order:
            order.append(I)
    order += [i for i in range(nchunks) if i not in order]
    return order


def _build_program(sched, slot_of, totcol, nchunks):
    import concourse.bass as bass
    import concourse.tile as tile
    from concourse import bacc, mybir

    f32 = mybir.dt.float32
    f16 = mybir.dt.float16

    nc = bacc.Bacc(
        "TRN2", target_bir_lowering=False, debug=False, num_devices=N_CORES
    )
    xt = nc.dram_tensor(
        "xt", [NGRP, 128, nchunks * GCOL], f16, kind="ExternalInput"
    ).ap()
    mop = nc.dram_tensor("mopb", [128, totcol], f16, kind="ExternalInput").ap()
    outt = nc.dram_tensor(
        "outt", [NGRP, 128, NTILE * GCOL], f16, kind="ExternalOutput"
    ).ap()

    # split points: xT pieces by first use (small first piece so the
    # first matmuls start early); mop likewise
    XSPLS = sorted(set([0, 2 * GCOL, 4 * GCOL, 7 * GCOL, 10 * GCOL,
                        14 * GCOL, nchunks * GCOL]))
    MSPLS = sorted(
        set([0] + [totcol * k // 6 // 2 * 2 for k in range(1, 6)] + [totcol])
    )
    OSPL = [0, 5 * GCOL, 9 * GCOL, 12 * GCOL, NTILE * GCOL]
    _emitted = set()

    with tile.TileContext(nc) as tc:
        with (
            tc.tile_pool(name="const", bufs=1) as const_pool,
            tc.tile_pool(name="xg", bufs=2) as xg_pool,
            tc.tile_pool(name="outp", bufs=2) as out_pool,
            tc.tile_pool(name="ps", bufs=7, space="PSUM") as ps_pool,
            tc.tile_pool(name="wm", bufs=1, space="PSUM") as warm_pool,
        ):
            # warmup: dummy matmuls on an uninitialized SBUF tile keep the
            # PE busy through the HAM activity window while the first DMA
            # pieces land, so the real matmuls run at 2.4 GHz from the
            # start instead of 1.2 GHz for their first ~10us
            wsrc = const_pool.tile([128, 512], f16, tag="warm")
            nc.gpsimd.memset(wsrc[:], 0)
            wps = warm_pool.tile([128, 512], f32, tag="wps")

            def dummy_mms(k):
                # keep the PE busy through startup DMA waits: heats the
                # HAM activity window and prevents a mid-start re-throttle
                for _ in range(k):
                    nc.tensor.matmul(
                        wps[:], wsrc[:, 0:128], wsrc[:],
                        start=True, stop=True, skip_group_check=True,
                    )

            dummy_mms(18)

            mop_sb = const_pool.tile([128, totcol], f16, tag="mop")
            for a, b in zip(MSPLS, MSPLS[1:]):
                nc.scalar.dma_start(mop_sb[:, a:b], mop[:, a:b])

            for g in range(NGRP):
                xg = xg_pool.tile([128, nchunks * GCOL], f16, tag="x")
                for a, b in zip(XSPLS, XSPLS[1:]):
                    nc.sync.dma_start(xg[:, a:b], xt[g, :, a:b])
                ot = out_pool.tile([128, NTILE * GCOL], f16, tag="o")
                ps_of = {}
                drained = set()
                for (j, off, w, rb, I, st, sp) in sched:
                    if j not in ps_of:
                        ps_of[j] = ps_pool.tile(
                            [128, GCOL], f32, tag="ps", name=f"ps_g{g}_t{j}"
                        )
                    s = slot_of[I] * GCOL
                    nc.tensor.matmul(
                        ps_of[j][rb:rb + w, :],
                        mop_sb[:, off:off + w],
                        xg[:, s:s + GCOL],
                        start=st,
                        stop=sp,
                        tile_position=(0, rb),
                        skip_group_check=True,
                    )
                    if sp:
                        dst = ot[:, j * GCOL:(j + 1) * GCOL]
                        if j % 2 == 0:
                            nc.vector.tensor_copy(dst, ps_of[j][:])
                        else:
                            nc.scalar.copy(dst, ps_of[j][:])
                        drained.add(j)
                        for a, b in zip(OSPL, OSPL[1:]):
                            if b // GCOL - 1 in drained and all(
                                t in drained for t in range(a // GCOL, b // GCOL)
                            ) and (a, b, g) not in _emitted:
                                _emitted.add((a, b, g))
                                nc.sync.dma_start(
                                    outt[g, :, a:b], ot[:, a:b]
                                )
                        if g == 0 and len(drained) in (3, 6):
                            dummy_mms(2)

    nc.compile()
    return nc


def kernel(x, edge_index, W1, W2, W3, b1, b2, b3):
    from concourse.bass_utils import run_bass_kernel_spmd

    x = np.asarray(x, dtype=np.float32)
    edge_index = np.asarray(edge_index)
    Ws = [np.asarray(W, dtype=np.float64) for W in (W1, W2, W3)]
    bs = [np.asarray(b, dtype=np.float64) for b in (b1, b2, b3)]

    As = [_dense_adj(edge_index[k]) for k in range(3)]
    plan = _plan(As)
    mopb, sched = _mop_blocks(plan, As, Ws)
    nchunks = len(plan["chunks"])
    order = _chunk_order(sched, nchunks)
    slot_of = {I: s for s, I in enumerate(order)}
    totcol = mopb.shape[1]

    key = str(sched) + str(order) + str(nchunks)
    if _PROGRAM_CACHE.get("key") != key:
        _PROGRAM_CACHE["nc"] = _build_program(sched, slot_of, totcol, nchunks)
        _PROGRAM_CACHE["key"] = key
    nc = _PROGRAM_CACHE["nc"]

    # host-side prep: fp16 cast + transpose + chunk packing in slot
    # (first-use) order; hot sources may appear in several chunks
    x16 = x.astype(np.float16)
    xr = x16.reshape(N_CORES, NGRP, GCOL, NNODES, C).transpose(0, 1, 3, 4, 2)
    xr = np.concatenate(
        [xr, np.zeros((N_CORES, NGRP, 1, C, GCOL), dtype=np.float16)], axis=2
    )  # zero-pad node index 25
    pidx = np.array(
        [
            (m if m is not None else NNODES)
            for I in order
            for m in plan["chunks"][I]
        ]
    )
    # [cores, grp, 26, 64, gcol] -> [cores, grp, 128(slot-major), ...]
    xtil = xr[:, :, pidx].reshape(N_CORES, NGRP, nchunks, 128, GCOL)
    xtil = np.ascontiguousarray(
        xtil.transpose(0, 1, 3, 2, 4).reshape(
            N_CORES, NGRP, 128, nchunks * GCOL
        )
    )

    in_maps = [{"xt": xtil[i], "mopb": mopb} for i in range(N_CORES)]
    res = run_bass_kernel_spmd(nc, in_maps, list(range(N_CORES)), **_RUN_KW)
    _PROGRAM_CACHE["last_result"] = res

    bias = np.zeros(C, dtype=np.float64)
    for b in bs:
        bias += b
    out = np.empty((N_CORES, BT_LOC, NNODES, C), dtype=np.float32)
    for i in range(N_CORES):
        # [grp, 128, NTILE*gcol] -> [grp, 128, NTILE, gcol]
        ot = (
            res.results[i]["outt"]
            .reshape(NGRP, 128, NTILE, GCOL)
            .astype(np.float32)
        )
        for j, t in enumerate(plan["tiles"]):
            out[i, :, t["ntop"]] = (
                ot[:, 0:C, j].transpose(0, 2, 1).reshape(BT_LOC, C)
            )
            if t["nbot"] is not None:
                out[i, :, t["nbot"]] = (
                    ot[:, C:128, j].transpose(0, 2, 1).reshape(BT_LOC, C)
                )
    out += bias.astype(np.float32)
    return np.ascontiguousarray(
        out.reshape(B, T, NNODES, C).astype(np.float32)
    )


# revision 28
# speedup vs baseline: 1.0853x; 1.0334x over previous
"""Trainium2 Bass kernel for a 3-branch GCN layer (sum of three GCNConvs).

Math: out[b,t] = sum_k A_k @ (x[b,t] @ W_k) + b_k with A_k the normalized
adjacency (self loops) of tiny shared graphs, so the operator collapses to
one [1600 x 1600] block matrix Mop = sum_k kron(A_k^T, W_k) applied to x
rows. Mop is BLOCK-SPARSE: 64x64 block (m,n) is nonzero only when some
graph couples source node m to target node n (~30% density here).

Device-side design (data-parallel over batch across 8 cores):
  - x is cast to fp16 AND pre-transposed/tiled on the host into xT chunks
    [128 (m-pair, d), 480 (bt)] so the device does ZERO transposes.
  - Flipped GEMM: out^T[(n,c), bt] accumulates in PSUM; the Mop blocks are
    PE-stationary, xT chunks stream 480 bt-columns (hides LDWEIGHTS). Only
    NONZERO blocks are stored/loaded/streamed.
  - Output nodes are matched into fixed pairs sharing one [128, 480] PSUM
    tile. A chunk coupling BOTH nodes of a pair runs as one full-width
    M=128 matmul; single-coupled chunks run as M=64 matmuls, interleaved
    top/bottom so PE column tiling ((0,0)/(0,64)) executes two at once.
  - Source m-pairs and target n-pairs are jointly optimized (matching) to
    minimize nonzero blocks. Host un-permutes out^T and adds bias.
  - DMA is batched: one xT load + one Mop piece + two out stores per
    group, so DGE issue cost (~0.6us each) stays off the critical path.
"""

import itertools
import sys

import numpy as np

if "/opt/trn_rl_repo" not in sys.path:
    sys.path.insert(0, "/opt/trn_rl_repo")

B, T, NNODES, C = 64, 300, 25, 64
F = NNODES * C  # 1600
N_CORES = 8
BT_LOC = (B // N_CORES) * T  # 2400
NGRP = 5
GCOL = BT_LOC // NGRP  # 480 bt-columns per group
NPAIR = 13  # 25 nodes + 1 zero-pad node -> 13 m-pair chunks of K=128
NTILE = 13  # 12 n-pairs + 1 single-n psum tile

_PROGRAM_CACHE = {}
# extra kwargs for run_bass_kernel_spmd (test harness sets trace=True here)
_RUN_KW = {}


def _dense_adj(edge_index_k: np.ndarray) -> np.ndarray:
    """PyG GCNConv normalized dense adjacency A[dst, src] (float64)."""
    row = edge_index_k[0].astype(np.int64)
    col = edge_index_k[1].astype(np.int64)
    loop = np.arange(NNODES, dtype=np.int64)
    row = np.concatenate([row, loop])
    col = np.concatenate([col, loop])
    deg = np.zeros(NNODES, dtype=np.float64)
    np.add.at(deg, col, 1.0)
    dinv = np.where(deg > 0, 1.0 / np.sqrt(deg), 0.0)
    norm = dinv[row] * dinv[col]
    A = np.zeros((NNODES, NNODES), dtype=np.float64)
    np.add.at(A, (col, row), norm)
    return A


def _match(items, weight):
    """Max-weight perfect-ish matching; networkx blossom with greedy
    fallback. Returns (pairs, leftover_single)."""
    items = list(items)
    try:
        import networkx as nx

        G = nx.Graph()
        G.add_nodes_from(items)
        for a, b in itertools.combinations(items, 2):
            G.add_edge(a, b, weight=weight(a, b))
        M = nx.max_weight_matching(G, maxcardinality=True)
        pairs = [tuple(sorted(p)) for p in M]
    except Exception:
        rem = set(items)
        pairs = []
        while len(rem) > 1:
            rl = sorted(rem)
            best, bv = None, -(10**9)
            for i, a in enumerate(rl):
                for b in rl[i + 1:]:
                    v = weight(a, b)
                    if v > bv:
                        bv, best = v, (a, b)
            pairs.append(best)
            rem -= set(best)
    used = set(x for p in pairs for x in p)
    single = [x for x in items if x not in used]
    return sorted(pairs), (single[0] if single else None)


def _cover(n, srcs_n, chunks):
    """Min chunks covering target n's source set: matched pairs (both
    sources in one chunk) + leftover singles. Returns {chunk_idx:
    set(active sources)}."""
    usable = {}
    for i, (a, b) in enumerate(chunks):
        if b is not None and a in srcs_n and b in srcs_n:
            usable.setdefault((a, b), i)
    try:
        import networkx as nx

        G = nx.Graph()
        G.add_nodes_from(srcs_n)
        for (a, b) in usable:
            G.add_edge(a, b)
        M = [tuple(sorted(p)) for p in nx.max_weight_matching(G)]
    except Exception:
        M = []
        rem = set(srcs_n)
        for (a, b) in sorted(usable):
            if a in rem and b in rem:
                M.append((a, b))
                rem -= {a, b}
    cov = {}
    used = set()
    for (a, b) in M:
        cov[usable[(a, b)]] = {a, b}
        used |= {a, b}
    for m in sorted(set(srcs_n) - used):
        ci = next(i for i, c in enumerate(chunks) if m in (c[0], c[1]))
        cov.setdefault(ci, set()).add(m)
    return cov


def _plan(As, max_chunks=21):
    """Choose source-pair chunks (hot sources may repeat across chunks),
    per-target covers, and target n-pairs, minimizing PE windows."""
    U = np.zeros((NNODES, NNODES), dtype=bool)
    for A in As:
        U |= A != 0.0  # U[n, m]: target n couples to source m
    srcs = [set(np.where(U[n])[0]) for n in range(NNODES)]

    # base chunks: matching maximizing shared target sets
    cols = [set(np.where(U[:, m])[0]) for m in range(NNODES)]
    mpairs, msingle = _match(range(NNODES), lambda a, b: len(cols[a] & cols[b]))
    chunks = mpairs + [(msingle, None)]

    # greedily duplicate hot source pairs into extra chunks while it
    # keeps reducing per-target coverage (fewer matmul windows); only
    # targets containing both candidate sources can improve
    covn = [len(_cover(n, srcs[n], chunks)) for n in range(NNODES)]
    while len(chunks) < max_chunks:
        best, bv = None, 0
        for a, b in itertools.combinations(range(NNODES), 2):
            if (a, b) in chunks:
                continue
            aff = [n for n in range(NNODES) if a in srcs[n] and b in srcs[n]]
            gain = sum(
                covn[n] - len(_cover(n, srcs[n], chunks + [(a, b)]))
                for n in aff
            )
            if gain > bv:
                bv, best = gain, (a, b)
        if best is None or bv < 2:
            break
        chunks.append(best)
        covn = [len(_cover(n, srcs[n], chunks)) for n in range(NNODES)]

    covs = [_cover(n, srcs[n], chunks) for n in range(NNODES)]
    rows = [set(c.keys()) for c in covs]

    # n-pairing: tile window count = max(|Ra|,|Rb|); overlap tie-break
    npairs, nsingle = _match(
        range(NNODES),
        lambda a, b: -3 * max(len(rows[a]), len(rows[b]))
        + len(rows[a] & rows[b]),
    )
    ncl = npairs + [(nsingle, None)]

    tiles = []
    tdelta = 0  # running (top singles - bottom singles)
    for na, nb in ncl:
        ra = rows[na]
        rb = rows[nb] if nb is not None else set()
        sa = sorted(ra - rb)
        sb = sorted(rb - ra)
        both = sorted(ra & rb)
        if nb is not None and (
            (tdelta > 0 and len(sa) > len(sb))
            or (tdelta < 0 and len(sb) > len(sa))
        ):
            na, nb, sa, sb = nb, na, sb, sa
        tdelta += len(sa) - len(sb)
        tiles.append({"ntop": na, "nbot": nb, "both": both, "top": sa, "bot": sb})
    return {"chunks": chunks, "covs": covs, "tiles": tiles, "U": U}


def _mop_blocks(plan, As, Ws):
    """Pack nonzero Mop blocks fp16 and build the matmul schedule.

    Returns (mopb [128, TOTCOL] f16, sched) with sched[j] = list of
    (col_off, width, row_base, chunk_idx, start, stop)."""
    Wstack = np.stack(Ws)  # [3, 64, 64] float64

    def half_block(n, I):
        # only sources ASSIGNED to chunk I for target n contribute; a
        # source present in the chunk but covered elsewhere stays zero
        blk = np.zeros((128, C), dtype=np.float64)
        active = plan["covs"][n].get(I, set())
        for h, m in enumerate(plan["chunks"][I]):
            if m is not None and m in active:
                coef = np.array([A[n, m] for A in As])
                blk[h * C:(h + 1) * C] = np.tensordot(coef, Wstack, 1)
        return blk

    cols = []
    off = 0
    pertile = []
    for t in plan["tiles"]:
        boths = []
        for I in t["both"]:
            cols.append(half_block(t["ntop"], I))
            cols.append(half_block(t["nbot"], I))
            boths.append((off, 128, 0, I))
            off += 128
        tops, bots = [], []
        for base, key, lst in ((0, "top", tops), (C, "bot", bots)):
            for I in t[key]:
                cols.append(half_block(t["ntop"] if base == 0 else t["nbot"], I))
                lst.append((off, C, base, I))
                off += C
        pertile.append((boths, tops, bots))
    mopb = np.concatenate(cols, axis=1).astype(np.float16)

    # Global schedule: flat instruction stream of (tile_j, off, w, rb, I).
    # Within a tile, top/bottom singles interleave (PE column tiling runs
    # the pair concurrently); LEFTOVER solos are carried forward and
    # paired with the next tiles' opposite-side solos across tile
    # boundaries — different psum banks + different column groups still
    # run concurrently.
    stream = []
    pend = []  # carried solos: list of (j, entry)
    last_kind = "b"
    for j, (boths, tops, bots) in enumerate(pertile):
        tops = [(j, e) for e in tops]
        bots = [(j, e) for e in bots]
        # pair carried solos with our opposite side first
        mine = {0: tops, C: bots}
        pairs = []
        for (pj, pe) in list(pend):
            other = mine[C if pe[2] == 0 else 0]
            if other:
                pairs.append(((pj, pe), other.pop(0)))
                pend.remove((pj, pe))
        inner = [p for p in itertools.zip_longest(mine[0], mine[C])
                 if p[0] is not None and p[1] is not None]
        lead = [x for pr in (pairs + inner) for x in pr]
        new_solo = [x for x in mine[0] + mine[C]
                    if x not in [e for pr in inner for e in pr]]
        bj = [(j, e) for e in boths]
        if last_kind == "b":
            seq = bj + lead
        else:
            seq = lead + bj
        if seq:
            last_kind = "b" if seq[-1][1][1] == 128 else "s"
        stream.extend(seq)
        pend.extend(new_solo)
        # flush carried solos that can no longer pair (keep at most 4)
        while len(pend) > 4:
            stream.append(pend.pop(0))
    stream.extend(pend)

    # start/stop flags per (tile, region); drains wait on each tile's
    # final entry, so record per-tile order
    seen = {}
    lastidx = {}
    for q, (j, (o, w, rb, I)) in enumerate(stream):
        lastidx[j] = q
    sched = []
    for q, (j, (o, w, rb, I)) in enumerate(stream):
        regions = ((j, 0), (j, 1)) if w == 128 else (
            ((j, 0),) if rb == 0 else ((j, 1),)
        )
        start = any(r not in seen for r in regions)
        for r in regions:
            seen[r] = True
        sched.append((j, o, w, rb, I, start, q == lastidx[j]))
    return np.ascontiguousarray(mopb), sched


def _chunk_order(sched, nchunks):
    """Chunk slots ordered by first use, so the first xT DMA piece covers
    the chunks the early tiles need."""
    order = []
    for (_, _, _, _, I, _, _) in sched:
        if I not in order:
            order.append(I)
    order += [i for i in range(nchunks) if i not in order]
    return order


def _build_program(sched, slot_of, totcol, nchunks):
    import concourse.bass as bass
    import concourse.tile as tile
    from concourse import bacc, mybir

    f32 = mybir.dt.float32
    f16 = mybir.dt.float16

    nc = bacc.Bacc(
        "TRN2", target_bir_lowering=False, debug=False, num_devices=N_CORES
    )
    xt = nc.dram_tensor(
        "xt", [NGRP, 128, nchunks * GCOL], f16, kind="ExternalInput"
    ).ap()
    mop = nc.dram_tensor("mopb", [128, totcol], f16, kind="ExternalInput").ap()
    outt = nc.dram_tensor(
        "outt", [NGRP, 128, NTILE * GCOL], f16, kind="ExternalOutput"
    ).ap()

    # split points: xT pieces by first use (small first piece so the
    # first matmuls start early); mop likewise
    XSPLS = sorted(set([0, 2 * GCOL, 4 * GCOL, 7 * GCOL, 10 * GCOL,
                        14 * GCOL, nchunks * GCOL]))
    MSPLS = sorted(
        set([0] + [totcol * k // 6 // 2 * 2 for k in range(1, 6)] + [totcol])
    )
    OSPL = [0, 5 * GCOL, 9 * GCOL, 12 * GCOL, NTILE * GCOL]
    _emitted = set()

    with tile.TileContext(nc) as tc:
        with (
            tc.tile_pool(name="const", bufs=1) as const_pool,
            tc.tile_pool(name="xg", bufs=2) as xg_pool,
            tc.tile_pool(name="outp", bufs=2) as out_pool,
            tc.tile_pool(name="ps", bufs=7, space="PSUM") as ps_pool,
            tc.tile_pool(name="wm", bufs=1, space="PSUM") as warm_pool,
        ):
            # warmup: dummy matmuls on an uninitialized SBUF tile keep the
            # PE busy through the HAM activity window while the first DMA
            # pieces land, so the real matmuls run at 2.4 GHz from the
            # start instead of 1.2 GHz for their first ~10us
            wsrc = const_pool.tile([128, 512], f16, tag="warm")
            nc.gpsimd.memset(wsrc[:], 0)
            wps = warm_pool.tile([128, 512], f32, tag="wps")

            def dummy_mms(k):
                # keep the PE busy through startup DMA waits: heats the
                # HAM activity window and prevents a mid-start re-throttle
                for _ in range(k):
                    nc.tensor.matmul(
                        wps[:], wsrc[:, 0:128], wsrc[:],
                        start=True, stop=True, skip_group_check=True,
                    )

            dummy_mms(20)

            mop_sb = const_pool.tile([128, totcol], f16, tag="mop")
            for a, b in zip(MSPLS, MSPLS[1:]):
                nc.scalar.dma_start(mop_sb[:, a:b], mop[:, a:b])

            for g in range(NGRP):
                xg = xg_pool.tile([128, nchunks * GCOL], f16, tag="x")
                for a, b in zip(XSPLS, XSPLS[1:]):
                    nc.sync.dma_start(xg[:, a:b], xt[g, :, a:b])
                ot = out_pool.tile([128, NTILE * GCOL], f16, tag="o")
                ps_of = {}
                drained = set()
                for (j, off, w, rb, I, st, sp) in sched:
                    if j not in ps_of:
                        ps_of[j] = ps_pool.tile(
                            [128, GCOL], f32, tag="ps", name=f"ps_g{g}_t{j}"
                        )
                    s = slot_of[I] * GCOL
                    nc.tensor.matmul(
                        ps_of[j][rb:rb + w, :],
                        mop_sb[:, off:off + w],
                        xg[:, s:s + GCOL],
                        start=st,
                        stop=sp,
                        tile_position=(0, rb),
                        skip_group_check=True,
                    )
                    if sp:
                        dst = ot[:, j * GCOL:(j + 1) * GCOL]
                        if j % 2 == 0:
                            nc.vector.tensor_copy(dst, ps_of[j][:])
                        else:
                            nc.scalar.copy(dst, ps_of[j][:])
                        drained.add(j)
                        # out stores go on the vector/scalar queues: the
                        # sync queue's serialized transfers are near the
                        # per-group budget with the xT loads alone
                        for pi, (a, b) in enumerate(zip(OSPL, OSPL[1:])):
                            if b // GCOL - 1 in drained and all(
                                t in drained for t in range(a // GCOL, b // GCOL)
                            ) and (a, b, g) not in _emitted:
                                _emitted.add((a, b, g))
                                nc.scalar.dma_start(
                                    outt[g, :, a:b], ot[:, a:b]
                                )
                        if g == 0 and len(drained) in (3, 6):
                            dummy_mms(2)

    nc.compile()
    return nc


def kernel(x, edge_index, W1, W2, W3, b1, b2, b3):
    from concourse.bass_utils import run_bass_kernel_spmd

    x = np.asarray(x, dtype=np.float32)
    edge_index = np.asarray(edge_index)
    Ws = [np.asarray(W, dtype=np.float64) for W in (W1, W2, W3)]
    bs = [np.asarray(b, dtype=np.float64) for b in (b1, b2, b3)]

    As = [_dense_adj(edge_index[k]) for k in range(3)]
    plan = _plan(As)
    mopb, sched = _mop_blocks(plan, As, Ws)
    nchunks = len(plan["chunks"])
    order = _chunk_order(sched, nchunks)
    slot_of = {I: s for s, I in enumerate(order)}
    totcol = mopb.shape[1]

    key = str(sched) + str(order) + str(nchunks)
    if _PROGRAM_CACHE.get("key") != key:
        _PROGRAM_CACHE["nc"] = _build_program(sched, slot_of, totcol, nchunks)
        _PROGRAM_CACHE["key"] = key
    nc = _PROGRAM_CACHE["nc"]

    # host-side prep: fp16 cast + transpose + chunk packing in slot
    # (first-use) order; hot sources may appear in several chunks
    x16 = x.astype(np.float16)
    xr = x16.reshape(N_CORES, NGRP, GCOL, NNODES, C).transpose(0, 1, 3, 4, 2)
    xr = np.concatenate(
        [xr, np.zeros((N_CORES, NGRP, 1, C, GCOL), dtype=np.float16)], axis=2
    )  # zero-pad node index 25
    pidx = np.array(
        [
            (m if m is not None else NNODES)
            for I in order
            for m in plan["chunks"][I]
        ]
    )
    # [cores, grp, 26, 64, gcol] -> [cores, grp, 128(slot-major), ...]
    xtil = xr[:, :, pidx].reshape(N_CORES, NGRP, nchunks, 128, GCOL)
    xtil = np.ascontiguousarray(
        xtil.transpose(0, 1, 3, 2, 4).reshape(
            N_CORES, NGRP, 128, nchunks * GCOL
        )
    )

    in_maps = [{"xt": xtil[i], "mopb": mopb} for i in range(N_CORES)]
    res = run_bass_kernel_spmd(nc, in_maps, list(range(N_CORES)), **_RUN_KW)
    _PROGRAM_CACHE["last_result"] = res

    bias = np.zeros(C, dtype=np.float64)
    for b in bs:
        bias += b
    out = np.empty((N_CORES, BT_LOC, NNODES, C), dtype=np.float32)
    for i in range(N_CORES):
        # [grp, 128, NTILE*gcol] -> [grp, 128, NTILE, gcol]
        ot = (
            res.results[i]["outt"]
            .reshape(NGRP, 128, NTILE, GCOL)
            .astype(np.float32)
        )
        for j, t in enumerate(plan["tiles"]):
            out[i, :, t["ntop"]] = (
                ot[:, 0:C, j].transpose(0, 2, 1).reshape(BT_LOC, C)
            )
            if t["nbot"] is not None:
                out[i, :, t["nbot"]] = (
                    ot[:, C:128, j].transpose(0, 2, 1).reshape(BT_LOC, C)
                )
    out += bias.astype(np.float32)
    return np.ascontiguousarray(
        out.reshape(B, T, NNODES, C).astype(np.float32)
    )
